# revision 1
# baseline (speedup 1.0000x reference)
"""Trainium2 Bass kernel for NodeCorrespondenceSelector (topk_masking).

Reference semantics: mask confidence <= 0.1 to zero, take the 256 SMALLEST
of the masked [B, N*M] map (top_k of the negation), unravel to (src, tgt).

Key property: ~10% of uniform entries are <= 0.1 and become exactly 0.0,
so the 256 smallest masked values are all 0.0 and XLA's stable top_k picks
them in ascending flat-index order.  The answer is therefore exactly the
first 256 flat indices with value <= 0.1 per batch row, ascending.  Those
all live in a short prefix of each row (the 256th hit sits near flat
position ~2560), so each core only needs to scan a 3584-element prefix.
The host verifies the device result is consistent (>= 256 hits in the
prefix, strictly increasing positions) and falls back to an exact host
computation otherwise (P(<256 hits in 3584) ~ 4e-9 per row).

Device algorithm per core (one batch row per core, 8 cores):
  1. mask m = (x <= 0.1) on a [128, 32] tile (flat order, partition-major)
  2. L = inclusive cumsum of m along the free dim (tensor_tensor_scan)
  3. per-partition totals t -> flat [1,129] -> inclusive scan of
     [0, t0..t127] = exclusive base offsets
  4. broadcast C(i) = L(i) + base(p(i)) to all 128 partitions via two
     accumulating rank-1 bf16 matmuls per 512-wide PSUM bank:
       ps[q, i] = 1*Lf[i] + 1*base[i/32]
     (bf16 rounding is safe: C values that matter for k <= 255 are < 256
     and exact in bf16; rounded larger values can never fall below 256)
  5. counts: out[k] = sum_i [C(i) <= k]  (= flat position of (k+1)-th hit)
       k =   0..127: VectorE  tensor_scalar(is_le, accum_out)
       k = 128..255: ScalarE  activation(Relu, bias=k+1, scale=-1,
                     accum_out) giving A(k) = sum_i relu(k+1 - C(i));
                     host takes adjacent differences (A(127) = sum of
                     the VectorE half).
     PSUM is split in two halves read in opposite order by the two
     engines so they run concurrently (same-tile reads serialize).
"""

import numpy as np

_THRES = np.float32(0.1)
_K = 256
_P = 128            # SBUF partitions
_F = 28             # free elements per partition in the prefix tile
_P2 = _P * _F       # 3584: prefix elements scanned on device per row
_H = _P2 // 2       # half width (one PSUM tile)
_NCORES = 8

_NC_CACHE = {}


def _build_nc():
    import concourse.bacc as bacc
    import concourse.mybir as mybir
    from concourse.tile import TileContext

    dt = mybir.dt
    op = mybir.AluOpType
    act = mybir.ActivationFunctionType

    nc = bacc.Bacc(trn_type="TRN2", debug=False, enable_asserts=False)
    x = nc.dram_tensor("x", [_P, _F], dt.float32, kind="ExternalInput")
    kvec = nc.dram_tensor("kvec", [_P, 2], dt.float32, kind="ExternalInput")
    tri = nc.dram_tensor("tri", [_P, _P], dt.bfloat16, kind="ExternalInput")
    cnt = nc.dram_tensor("cnt", [_P, 8], dt.float32, kind="ExternalOutput")

    with TileContext(nc) as tc:
        with (
            tc.tile_pool(name="sb", bufs=1) as pool,
            tc.tile_pool(name="ps", bufs=1, space="PSUM") as psum,
        ):
            xt = pool.tile([_P, _F], dt.float32, tag="xt")
            nc.sync.dma_start(xt[:], x[:, :])
            trit = pool.tile([_P, _P], dt.bfloat16, tag="trit")
            nc.scalar.dma_start(trit[:], tri[:, :])
            kv = pool.tile([_P, 2], dt.float32, tag="kv")
            nc.scalar.dma_start(kv[:], kvec[:, :])

            o2 = pool.tile([2, _P], dt.bfloat16, tag="o2")
            nc.vector.memset(o2[:2, :], 1.0)
            z = pool.tile([_P, _F], dt.float32, tag="z")
            nc.vector.memset(z[:], 0.0)

            m = pool.tile([_P, _F], dt.float32, tag="m")
            nc.vector.tensor_scalar(m[:], xt[:], float(_THRES), None, op.is_le)
            t = pool.tile([_P, 1], dt.bfloat16, tag="t")
            with nc.allow_low_precision(reason="counts <= 28 are exact in bf16"):
                nc.vector.tensor_reduce(
                    t[:], m[:], axis=mybir.AxisListType.X, op=op.add
                )
            L = pool.tile([_P, _F], dt.bfloat16, tag="L")
            nc.vector.tensor_tensor_scan(
                L[:], m[:], z[:], 0.0, op.add, op.add
            )

            # four PSUM tiles (PSUM dependency tracking is tile-granular,
            # so independent readers need separate tiles): 2+2+2+1 banks
            psA = psum.tile([_P, 1024], dt.float32, tag="psA")
            psB = psum.tile([_P, 1024], dt.float32, tag="psB")
            psC = psum.tile([_P, 1024], dt.float32, tag="psC")
            psD = psum.tile([_P, 512], dt.float32, tag="psD")
            psT = [psA, psB, psC, psD]
            psW = [1024, 1024, 1024, 512]

            # base[q] = sum_{p<q} t[p] via PE: lhsT = tri (tri[p, q] = 1 iff
            # p < q), rhs = t; lands in psA bank 0, which the broadcast
            # matmuls below overwrite afterwards.
            nc.tensor.matmul(
                psT[0][:, 0:1], trit[:], t[:], start=True, stop=True
            )
            # expand base to [128, _F] (per-partition broadcast)
            brep = pool.tile([_P, _F], dt.bfloat16, tag="brep")
            nc.vector.tensor_scalar(brep[:], z[:], psT[0][:, 0:1], None, op.add)

            # T2 row 0 = L flattened (partition-major) = L(i); row 1 = brep
            # flattened = base[i // 32]
            T2 = pool.tile([2, _P2], dt.bfloat16, tag="T2")
            nc.sync.dma_start(T2[:1, :], L[:])
            nc.scalar.dma_start(T2[1:2, :], brep[:])

            # ps[q, i] = Lf[i] + base[i // 32]  for all q
            npb = 512  # one PSUM bank of f32
            for b in range(_P2 // npb):
                sl = slice(b * npb, (b + 1) * npb)
                pst = psT[min(b // 2, 3)]
                off = b * npb - [0, 1024, 2048, 3072][min(b // 2, 3)]
                nc.tensor.matmul(
                    pst[:, off : off + npb], o2[:2, :], T2[:2, sl],
                    start=True, stop=True,
                )

            # counts over four quarters, engines staggered so both run
            # concurrently and start as soon as the relevant banks are done
            # (disjoint-slice reads don't serialize; same-slice reads do,
            # in emission order)
            G0 = pool.tile([_P, _P2], dt.float32, tag="G0")
            G1 = pool.tile([_P, _P2], dt.float32, tag="G1")
            S = pool.tile([_P, 8], dt.float32, tag="S")

            # DVE first-reads tiles 0 and 2; ACT first-reads tiles 1 and 3;
            # second round swapped, so both engines run concurrently.
            GOFF = [0, 1024, 2048, 3072]
            for eng, ti, col in (
                ("v", 0, 0), ("a", 1, 5), ("v", 2, 1), ("a", 3, 7),
                ("v", 1, 2), ("a", 0, 4), ("v", 3, 3), ("a", 2, 6),
            ):
                pst = psT[ti]
                w = psW[ti]
                sl = slice(GOFF[ti], GOFF[ti] + w)
                if eng == "v":
                    nc.vector.tensor_scalar(
                        G0[:, sl], pst[:, :], kv[:, 0:1], None,
                        op.is_le, op1=op.add, accum_out=S[:, col : col + 1],
                    )
                else:
                    nc.scalar.activation(
                        G1[:, sl], pst[:, :], act.Relu,
                        bias=kv[:, 1:2], scale=-1.0,
                        accum_out=S[:, col : col + 1],
                    )
            nc.sync.dma_start(cnt[:, :], S[:])
    nc.compile()
    return nc


def _get_nc():
    if "nc" not in _NC_CACHE:
        _NC_CACHE["nc"] = _build_nc()
    return _NC_CACHE["nc"]


def _make_kvec():
    # col 0: k values 0..127 for the VectorE is_le half
    # col 1: Relu biases k+1 = 129..256 for the ScalarE half (k = 128..255)
    kvec = np.empty((_P, 2), np.float32)
    kvec[:, 0] = np.arange(_P, dtype=np.float32)
    kvec[:, 1] = np.arange(_P, dtype=np.float32) + 129.0
    return kvec


def _decode_counts(cnt_out):
    """cnt_out: [128, 8] f32 from one core -> [256] int64 positions."""
    s0 = cnt_out[:, 0:4].astype(np.float64).sum(axis=1)
    s1 = cnt_out[:, 4:8].astype(np.float64).sum(axis=1)
    a_prev = np.concatenate([[s0.sum()], s1[:-1]])  # A(127..254)
    hi = s1 - a_prev
    return np.concatenate([s0, hi]).astype(np.int64)


def _run_device(prefix, trace=False):
    """prefix: [8, 4096] f32.  Returns (positions [8, 256] int64, results)."""
    import ml_dtypes
    from concourse.bass_utils import run_bass_kernel_spmd

    nc = _get_nc()
    kvec = _make_kvec()
    tri = np.triu(np.ones((_P, _P), np.float32), 1).astype(ml_dtypes.bfloat16)
    in_maps = [
        {
            "x": np.ascontiguousarray(prefix[c].reshape(_P, _F)),
            "kvec": kvec,
            "tri": tri,
        }
        for c in range(_NCORES)
    ]
    res = run_bass_kernel_spmd(
        nc, in_maps, core_ids=list(range(_NCORES)), trace=trace
    )
    pos = np.stack([_decode_counts(res.results[c]["cnt"]) for c in range(_NCORES)])
    return pos, res


def _host_row(flat_row):
    """Exact reference semantics for one row (fallback path)."""
    mask = flat_row <= _THRES
    hits = np.flatnonzero(mask)
    if hits.size >= _K:
        return hits[:_K].astype(np.int64)
    masked = np.where(flat_row > _THRES, flat_row, np.float32(0.0))
    order = np.argsort(masked, kind="stable")
    return order[:_K].astype(np.int64)


def kernel(confidence_map):
    cm = np.asarray(confidence_map)
    if cm.dtype != np.float32:
        cm = cm.astype(np.float32)
    B = cm.shape[0]
    num_tgt = cm.shape[2]
    flat = cm.reshape(B, -1)

    idx = None
    if B == _NCORES and flat.shape[1] >= _P2:
        pos, _ = _run_device(flat[:, :_P2])
        ok = bool(
            pos.min() >= 0
            and pos.max() < _P2
            and np.all(np.diff(pos, axis=1) > 0)
        )
        if ok:
            idx = pos
    if idx is None:
        idx = np.stack([_host_row(flat[b]) for b in range(B)])

    src = (idx // num_tgt).astype(np.int32)
    tgt = (idx % num_tgt).astype(np.int32)
    return np.stack([src, tgt], axis=-1)



# revision 2
# speedup vs baseline: 1.2132x; 1.2132x over previous
"""Trainium2 Bass kernel for NodeCorrespondenceSelector (topk_masking).

Reference semantics: mask confidence <= 0.1 to zero, take the 256 SMALLEST
of the masked [B, N*M] map (top_k of the negation), unravel to (src, tgt).

Key property: ~10% of uniform entries are <= 0.1 and become exactly 0.0,
so the 256 smallest masked values are all 0.0 and XLA's stable top_k picks
them in ascending flat-index order.  The answer is therefore exactly the
first 256 flat indices with value <= 0.1 per batch row, ascending.  Those
all live in a short prefix of each row: the 4096-element prefix holds
~410 +- 19 hits, so P(<256 hits) ~ 1e-15 per row.  The host verifies the
device result is consistent (>= 256 hits, integral block sums, strictly
increasing positions) and falls back to an exact host computation
otherwise.

Device algorithm per core (one batch row per core, 8 cores), coarse/fine
two-level counting over a [32 blocks x 128 lane] transposed layout
(xT[f, p] = prefix[f*128 + p], prepared host-side by a free reshape):

  1. mask   mT = (xT <= 0.1)                    [32, 128] bf16   (VectorE)
  2. scan   CT = within-block inclusive cumsum  [32, 128] bf16   (VectorE)
     (CT <= 128, exact in bf16)
  3. block prefix: colsum = CT[:, -1];  two tiny PE matmuls with
     triangular constants give bc_incl/bc_excl [32, 1] each (fp32 PSUM)
  4. Cfull  CfT = CT + bc_excl  (ScalarE Relu with per-partition bias;
     values > 256 round in bf16 but stay > 255, so every comparison
     against thresholds <= 255 is still exact)
  5. coarse: S[f, j] = (bc_excl[f] < j <= bc_incl[f]) -- one-hot of the
     block holding hit j -- built with two VectorE ops against a constant
     j-iota [32, 256]
  6. fine:  G = S^T @ CfT (two PE matmuls, K=32, N=128) gathers each
     hit's block column; r[j] = #{p: G[j, p] <= j-1} via two accumulating
     VectorE compares (j in 1..128 on col 0, 129..256 on col 1)
  7. outputs: rv [128, 2] and bc [32, 2]; host computes
     F(j) = #{f: bc_incl[f] <= j-1} from bc and pos = 128*F + r.
"""

import numpy as np

_THRES = np.float32(0.1)
_K = 256
_NB = 32            # blocks = SBUF partitions of the transposed layout
_BP = 128           # elements per block (free dim)
_P2 = _NB * _BP     # 4096: prefix elements scanned on device per row
_NCORES = 8

_NC_CACHE = {}


def _build_nc():
    import concourse.bacc as bacc
    import concourse.mybir as mybir
    from concourse.tile import TileContext

    dt = mybir.dt
    op = mybir.AluOpType
    act = mybir.ActivationFunctionType

    nc = bacc.Bacc(trn_type="TRN2", debug=False, enable_asserts=False)
    x = nc.dram_tensor("x", [_NB, _BP], dt.float32, kind="ExternalInput")
    # packed bf16 constants: cols 0:256 j-iota (1..128 | 129..256),
    # 256:288 triu-inclusive [32, 32], 288:320 triu-strict [32, 32]
    cst = nc.dram_tensor("cst", [_NB, 320], dt.bfloat16, kind="ExternalInput")
    kv = nc.dram_tensor("kv", [128, 2], dt.float32, kind="ExternalInput")
    outR = nc.dram_tensor("outR", [128, 2], dt.float32, kind="ExternalOutput")
    outB = nc.dram_tensor("outB", [_NB, 2], dt.float32, kind="ExternalOutput")

    with TileContext(nc) as tc:
        with (
            tc.tile_pool(name="sb", bufs=1) as pool,
            tc.tile_pool(name="ps", bufs=1, space="PSUM") as psum,
        ):
            xt = pool.tile([_NB, _BP], dt.float32, tag="xt")
            nc.sync.dma_start(xt[:], x[:, :])
            cs = pool.tile([_NB, 320], dt.bfloat16, tag="cs")
            nc.scalar.dma_start(cs[:], cst[:, :])
            kvt = pool.tile([128, 2], dt.float32, tag="kvt")
            nc.scalar.dma_start(kvt[:], kv[:, :])

            z = pool.tile([_NB, _BP], dt.bfloat16, tag="z")
            nc.gpsimd.memset(z[:], 0.0)

            # 1. mask
            mT = pool.tile([_NB, _BP], dt.bfloat16, tag="mT")
            nc.vector.tensor_scalar(mT[:], xt[:], float(_THRES), None, op.is_le)
            # 2. within-block inclusive scan (fp32 state, bf16 out, <=128)
            CT = pool.tile([_NB, _BP], dt.bfloat16, tag="CT")
            nc.vector.tensor_tensor_scan(
                CT[:], mT[:], z[:], 0.0, op.add, op.add
            )

            # 3. block prefix sums via triangular matmuls (N=1)
            psBC = psum.tile([_NB, 2], dt.float32, tag="psBC")
            nc.tensor.matmul(
                psBC[:, 0:1], cs[:, 256:288], CT[:, 127:128],
                start=True, stop=True,
            )
            nc.tensor.matmul(
                psBC[:, 1:2], cs[:, 288:320], CT[:, 127:128],
                start=True, stop=True,
            )
            bcsb = pool.tile([_NB, 2], dt.float32, tag="bcsb")
            nc.scalar.activation(bcsb[:], psBC[:], act.Copy)

            # 4. Cfull = CT + bc_excl (>=0, so Relu is the identity)
            CfT = pool.tile([_NB, _BP], dt.bfloat16, tag="CfT")
            nc.scalar.activation(
                CfT[:], CT[:], act.Relu, bias=bcsb[:, 1:2], scale=1.0
            )

            # 5. S[f, j] = (j > bc_excl[f]) * (j <= bc_incl[f])
            t1 = pool.tile([_NB, 256], dt.bfloat16, tag="t1")
            nc.vector.tensor_scalar(
                t1[:], cs[:, 0:256], bcsb[:, 1:2], None, op.is_gt
            )
            Sf = pool.tile([_NB, 256], dt.bfloat16, tag="Sf")
            nc.vector.scalar_tensor_tensor(
                Sf[:], cs[:, 0:256], bcsb[:, 0:1], t1[:], op.is_le, op.mult
            )

            # 6. G = S^T @ CfT, then fine counts
            psGlo = psum.tile([128, _BP], dt.float32, tag="psGlo")
            psGhi = psum.tile([128, _BP], dt.float32, tag="psGhi")
            nc.tensor.matmul(
                psGlo[:], Sf[:, 0:128], CfT[:], start=True, stop=True
            )
            nc.tensor.matmul(
                psGhi[:], Sf[:, 128:256], CfT[:], start=True, stop=True
            )

            rv = pool.tile([128, 2], dt.float32, tag="rv")
            dl = pool.tile([128, _BP], dt.float32, tag="dl")
            dh = pool.tile([128, _BP], dt.float32, tag="dh")
            nc.vector.tensor_scalar(
                dl[:], psGlo[:], kvt[:, 0:1], None,
                op.is_le, op1=op.add, accum_out=rv[:, 0:1],
            )
            nc.vector.tensor_scalar(
                dh[:], psGhi[:], kvt[:, 1:2], None,
                op.is_le, op1=op.add, accum_out=rv[:, 1:2],
            )

            nc.sync.dma_start(outR[:, :], rv[:])
            nc.scalar.dma_start(outB[:, :], bcsb[:])
    nc.compile()
    return nc


def _get_nc():
    if "nc" not in _NC_CACHE:
        _NC_CACHE["nc"] = _build_nc()
    return _NC_CACHE["nc"]


def _make_consts():
    import ml_dtypes

    cst = np.zeros((_NB, 320), np.float32)
    j = np.arange(1, 257, dtype=np.float32)
    cst[:, 0:256] = j[None, :]
    f = np.arange(_NB)
    cst[:, 256:288] = (f[:, None] <= f[None, :]).astype(np.float32)
    cst[:, 288:320] = (f[:, None] < f[None, :]).astype(np.float32)
    kv = np.empty((128, 2), np.float32)
    kv[:, 0] = np.arange(128, dtype=np.float32)
    kv[:, 1] = np.arange(128, dtype=np.float32) + 128.0
    return cst.astype(ml_dtypes.bfloat16), kv


def _decode_core(outR, outB):
    """outR: [128, 2] f32, outB: [32, 2] f32 -> ([256] int64, ok flag)."""
    bc_incl = outB[:, 0].astype(np.float64)
    r = np.concatenate([outR[:, 0], outR[:, 1]]).astype(np.float64)
    if not (
        np.all(bc_incl == np.floor(bc_incl))
        and np.all(np.diff(bc_incl) >= 0)
        and bc_incl[-1] >= _K
        and np.all(r == np.floor(r))
        and r.min() >= 0
        and r.max() <= _BP - 1
    ):
        return None
    jm1 = np.arange(_K, dtype=np.float64)
    F = np.searchsorted(bc_incl, jm1, side="right")
    if F.max() >= _NB:
        return None
    pos = (_BP * F + r).astype(np.int64)
    if not (np.all(np.diff(pos) > 0) and pos[0] >= 0 and pos[-1] < _P2):
        return None
    return pos


def _run_device(prefix, trace=False):
    """prefix: [8, 4096] f32.  Returns (positions [8, 256] or None, results)."""
    from concourse.bass_utils import run_bass_kernel_spmd

    nc = _get_nc()
    cst, kv = _make_consts()
    in_maps = [
        {
            "x": np.ascontiguousarray(prefix[c].reshape(_NB, _BP)),
            "cst": cst,
            "kv": kv,
        }
        for c in range(_NCORES)
    ]
    res = run_bass_kernel_spmd(
        nc, in_maps, core_ids=list(range(_NCORES)), trace=trace
    )
    pos = []
    for c in range(_NCORES):
        p = _decode_core(res.results[c]["outR"], res.results[c]["outB"])
        if p is None:
            return None, res
        pos.append(p)
    return np.stack(pos), res


def _host_row(flat_row):
    """Exact reference semantics for one row (fallback path)."""
    mask = flat_row <= _THRES
    hits = np.flatnonzero(mask)
    if hits.size >= _K:
        return hits[:_K].astype(np.int64)
    masked = np.where(flat_row > _THRES, flat_row, np.float32(0.0))
    order = np.argsort(masked, kind="stable")
    return order[:_K].astype(np.int64)


def kernel(confidence_map):
    cm = np.asarray(confidence_map)
    if cm.dtype != np.float32:
        cm = cm.astype(np.float32)
    B = cm.shape[0]
    num_tgt = cm.shape[2]
    flat = cm.reshape(B, -1)

    idx = None
    if B == _NCORES and flat.shape[1] >= _P2:
        idx, _ = _run_device(flat[:, :_P2])
    if idx is None:
        idx = np.stack([_host_row(flat[b]) for b in range(B)])

    src = (idx // num_tgt).astype(np.int32)
    tgt = (idx % num_tgt).astype(np.int32)
    return np.stack([src, tgt], axis=-1)


# revision 3
# speedup vs baseline: 1.3684x; 1.1279x over previous
"""Trainium2 Bass kernel for NodeCorrespondenceSelector (topk_masking).

Reference semantics: mask confidence <= 0.1 to zero, take the 256 SMALLEST
of the masked [B, N*M] map (top_k of the negation), unravel to (src, tgt).

Key property: ~10% of uniform entries are <= 0.1 and become exactly 0.0,
so the 256 smallest masked values are all 0.0 and XLA's stable top_k picks
them in ascending flat-index order.  The answer is therefore exactly the
first 256 flat indices with value <= 0.1 per batch row, ascending.  Those
all live in a short prefix of each row: the 4096-element prefix holds
~410 +- 19 hits, so P(<256 hits) ~ 1e-15 per row.  The host verifies the
device result is consistent (>= 256 hits, integral block sums, strictly
increasing positions) and falls back to an exact host computation
otherwise.

Device algorithm per core (one batch row per core, 8 cores), coarse/fine
two-level counting over a [32 blocks x 128 lane] transposed layout
(xT[f, p] = prefix[f*128 + p], prepared host-side by a free reshape):

  1. mask   mT = (xT <= 0.1)                    [32, 128] bf16   (VectorE)
  2. scan   CT = within-block inclusive cumsum  [32, 128] bf16   (VectorE)
     (CT <= 128, exact in bf16)
  3. block prefix: colsum = CT[:, -1];  two tiny PE matmuls with
     triangular constants give bc_incl/bc_excl [32, 1] each (fp32 PSUM)
  4. Cfull  CfT = CT + bc_excl  (ScalarE Relu with per-partition bias;
     values > 256 round in bf16 but stay > 255, so every comparison
     against thresholds <= 255 is still exact)
  5. coarse: S[f, j] = (bc_excl[f] < j <= bc_incl[f]) -- one-hot of the
     block holding hit j -- built with two VectorE ops against a constant
     j-iota [32, 256]
  6. fine:  G = S^T @ CfT (two PE matmuls, K=32, N=128) gathers each
     hit's block column; the two 128-wide count halves run in parallel:
       lo (j=1..128):   VectorE  is_le(G_lo, q) with free-dim accumulate
       hi (j=129..256): ScalarE  Sign(q+128.5 - G_hi) accumulated; the
                        count is (A+128)/2 since G is integral
  7. everything is PE-transposed onto partition 0 and shipped with a
     single [1, 288] f32 DMA (one descriptor per SDMA engine -- a
     [128, x] output tile pays ~3.4 us of per-descriptor completion
     trickle on the final semaphore); host computes
     F(j) = #{f: bc_incl[f] <= j-1} and pos = 128*F + r.
"""

import numpy as np

_THRES = np.float32(0.1)
_K = 256
_NB = 32            # blocks = SBUF partitions of the transposed layout
_BP = 128           # elements per block (free dim)
_P2 = _NB * _BP     # 4096: prefix elements scanned on device per row
_NCORES = 8

_NC_CACHE = {}


def _build_nc():
    import concourse.bacc as bacc
    import concourse.mybir as mybir
    from concourse.tile import TileContext

    dt = mybir.dt
    op = mybir.AluOpType
    act = mybir.ActivationFunctionType

    nc = bacc.Bacc(trn_type="TRN2", debug=False, enable_asserts=False)
    x = nc.dram_tensor("x", [_NB, _BP], dt.float32, kind="ExternalInput")
    # packed bf16 constants: cols 0:256 j-iota (1..128 | 129..256),
    # 256:288 triu-inclusive [32, 32], 288:320 triu-strict [32, 32]
    cst = nc.dram_tensor("cst", [_NB, 320], dt.bfloat16, kind="ExternalInput")
    kv = nc.dram_tensor("kv", [128, 2], dt.float32, kind="ExternalInput")
    idf = nc.dram_tensor("idf", [128, 128], dt.float32, kind="ExternalInput")
    outZ = nc.dram_tensor("outZ", [1, 288], dt.float32, kind="ExternalOutput")

    with TileContext(nc) as tc:
        with (
            tc.tile_pool(name="sb", bufs=1) as pool,
            tc.tile_pool(name="ps", bufs=1, space="PSUM") as psum,
        ):
            xt = pool.tile([_NB, _BP], dt.float32, tag="xt")
            nc.sync.dma_start(xt[0:16, :], x[0:16, :])
            nc.scalar.dma_start(xt[16:32, :], x[16:32, :])
            cs = pool.tile([_NB, 320], dt.bfloat16, tag="cs")
            nc.scalar.dma_start(cs[:], cst[:, :])
            kvt = pool.tile([128, 2], dt.float32, tag="kvt")
            nc.scalar.dma_start(kvt[:], kv[:, :])
            idt = pool.tile([128, 128], dt.float32, tag="idt")
            nc.sync.dma_start(idt[:], idf[:, :])

            z = pool.tile([_NB, _BP], dt.bfloat16, tag="z")
            nc.gpsimd.memset(z[:], 0.0)

            # 1. mask
            mT = pool.tile([_NB, _BP], dt.bfloat16, tag="mT")
            nc.vector.tensor_scalar(mT[:], xt[:], float(_THRES), None, op.is_le)
            # 2. within-block inclusive scan (fp32 state, bf16 out, <=128)
            CT = pool.tile([_NB, _BP], dt.bfloat16, tag="CT")
            nc.vector.tensor_tensor_scan(
                CT[:], mT[:], z[:], 0.0, op.add, op.add
            )

            # 3. block prefix sums via triangular matmuls (N=1)
            psBC = psum.tile([_NB, 2], dt.float32, tag="psBC")
            nc.tensor.matmul(
                psBC[:, 0:1], cs[:, 256:288], CT[:, 127:128],
                start=True, stop=True,
            )
            nc.tensor.matmul(
                psBC[:, 1:2], cs[:, 288:320], CT[:, 127:128],
                start=True, stop=True,
            )
            bcsb = pool.tile([_NB, 2], dt.float32, tag="bcsb")
            nc.scalar.activation(bcsb[:], psBC[:], act.Copy)

            # transposed outputs accumulate on partition 0 in one PSUM bank
            psZ = psum.tile([1, 512], dt.float32, tag="psZ")
            nc.tensor.transpose(psZ[0:1, 256:288], bcsb[:, 0:1], idt[0:_NB, 0:_NB])

            # 4. Cfull = CT + bc_excl (>=0, so Relu is the identity)
            CfT = pool.tile([_NB, _BP], dt.bfloat16, tag="CfT")
            nc.scalar.activation(
                CfT[:], CT[:], act.Relu, bias=bcsb[:, 1:2], scale=1.0
            )

            # 5. S[f, j] = (j > bc_excl[f]) * (j <= bc_incl[f])
            t1 = pool.tile([_NB, 256], dt.bfloat16, tag="t1")
            nc.vector.tensor_scalar(
                t1[:], cs[:, 0:256], bcsb[:, 1:2], None, op.is_gt
            )
            Sf = pool.tile([_NB, 256], dt.bfloat16, tag="Sf")
            nc.vector.scalar_tensor_tensor(
                Sf[:], cs[:, 0:256], bcsb[:, 0:1], t1[:], op.is_le, op.mult
            )

            # 6. G = S^T @ CfT, then the two fine-count halves in parallel
            psGlo = psum.tile([128, _BP], dt.float32, tag="psGlo")
            psGhi = psum.tile([128, _BP], dt.float32, tag="psGhi")
            nc.tensor.matmul(
                psGlo[:], Sf[:, 0:128], CfT[:], start=True, stop=True
            )
            nc.tensor.matmul(
                psGhi[:], Sf[:, 128:256], CfT[:], start=True, stop=True
            )

            rvlo = pool.tile([128, 1], dt.float32, tag="rvlo")
            rvA = pool.tile([128, 1], dt.float32, tag="rvA")
            dl = pool.tile([128, _BP], dt.float32, tag="dl")
            dh = pool.tile([128, _BP], dt.float32, tag="dh")
            nc.vector.tensor_scalar(
                dl[:], psGlo[:], kvt[:, 0:1], None,
                op.is_le, op1=op.add, accum_out=rvlo[:],
            )
            nc.scalar.activation(
                dh[:], psGhi[:], act.Sign,
                bias=kvt[:, 1:2], scale=-1.0, accum_out=rvA[:],
            )

            # 7. transpose counts onto partition 0; single tiny DMA out
            nc.tensor.transpose(psZ[0:1, 0:128], rvlo[:], idt[:, :])
            nc.tensor.transpose(psZ[0:1, 128:256], rvA[:], idt[:, :])
            zsb = pool.tile([1, 288], dt.float32, tag="zsb")
            nc.scalar.activation(zsb[:], psZ[0:1, 0:288], act.Copy)
            nc.sync.dma_start(outZ[:, :], zsb[:])
    nc.compile()
    return nc


def _get_nc():
    if "nc" not in _NC_CACHE:
        _NC_CACHE["nc"] = _build_nc()
    return _NC_CACHE["nc"]


def _make_consts():
    import ml_dtypes

    cst = np.zeros((_NB, 320), np.float32)
    j = np.arange(1, 257, dtype=np.float32)
    cst[:, 0:256] = j[None, :]
    f = np.arange(_NB)
    cst[:, 256:288] = (f[:, None] <= f[None, :]).astype(np.float32)
    cst[:, 288:320] = (f[:, None] < f[None, :]).astype(np.float32)
    kv = np.empty((128, 2), np.float32)
    kv[:, 0] = np.arange(128, dtype=np.float32)
    kv[:, 1] = np.arange(128, dtype=np.float32) + 128.5
    idf = np.eye(128, dtype=np.float32)
    return cst.astype(ml_dtypes.bfloat16), kv, idf


def _decode_core(z):
    """z: [1, 288] f32 -> [256] int64 positions, or None if inconsistent."""
    z = z.reshape(-1).astype(np.float64)
    r_lo = z[0:128]
    r_hi = (z[128:256] + 128.0) / 2.0
    bc_incl = z[256:288]
    r = np.concatenate([r_lo, r_hi])
    if not (
        np.all(bc_incl == np.floor(bc_incl))
        and np.all(np.diff(bc_incl) >= 0)
        and bc_incl[-1] >= _K
        and np.all(r == np.floor(r))
        and r.min() >= 0
        and r.max() <= _BP - 1
    ):
        return None
    jm1 = np.arange(_K, dtype=np.float64)
    F = np.searchsorted(bc_incl, jm1, side="right")
    if F.max() >= _NB:
        return None
    pos = (_BP * F + r).astype(np.int64)
    if not (np.all(np.diff(pos) > 0) and pos[0] >= 0 and pos[-1] < _P2):
        return None
    return pos


def _run_device(prefix, trace=False):
    """prefix: [8, 4096] f32.  Returns (positions [8, 256] or None, results)."""
    from concourse.bass_utils import run_bass_kernel_spmd

    nc = _get_nc()
    cst, kv, idf = _make_consts()
    in_maps = [
        {
            "x": np.ascontiguousarray(prefix[c].reshape(_NB, _BP)),
            "cst": cst,
            "kv": kv,
            "idf": idf,
        }
        for c in range(_NCORES)
    ]
    res = run_bass_kernel_spmd(
        nc, in_maps, core_ids=list(range(_NCORES)), trace=trace
    )
    pos = []
    for c in range(_NCORES):
        p = _decode_core(res.results[c]["outZ"])
        if p is None:
            return None, res
        pos.append(p)
    return np.stack(pos), res


def _host_row(flat_row):
    """Exact reference semantics for one row (fallback path)."""
    mask = flat_row <= _THRES
    hits = np.flatnonzero(mask)
    if hits.size >= _K:
        return hits[:_K].astype(np.int64)
    masked = np.where(flat_row > _THRES, flat_row, np.float32(0.0))
    order = np.argsort(masked, kind="stable")
    return order[:_K].astype(np.int64)


def kernel(confidence_map):
    cm = np.asarray(confidence_map)
    if cm.dtype != np.float32:
        cm = cm.astype(np.float32)
    B = cm.shape[0]
    num_tgt = cm.shape[2]
    flat = cm.reshape(B, -1)

    idx = None
    if B == _NCORES and flat.shape[1] >= _P2:
        idx, _ = _run_device(flat[:, :_P2])
    if idx is None:
        idx = np.stack([_host_row(flat[b]) for b in range(B)])

    src = (idx // num_tgt).astype(np.int32)
    tgt = (idx % num_tgt).astype(np.int32)
    return np.stack([src, tgt], axis=-1)


# revision 5
# speedup vs baseline: 1.3897x; 1.0156x over previous
"""Trainium2 Bass kernel for NodeCorrespondenceSelector (topk_masking).

Reference semantics: mask confidence <= 0.1 to zero, take the 256 SMALLEST
of the masked [B, N*M] map (top_k of the negation), unravel to (src, tgt).

Key property: ~10% of uniform entries are <= 0.1 and become exactly 0.0,
so the 256 smallest masked values are all 0.0 and XLA's stable top_k picks
them in ascending flat-index order.  The answer is therefore exactly the
first 256 flat indices with value <= 0.1 per batch row, ascending.  Those
all live in a short prefix of each row: the 4096-element prefix holds
~410 +- 19 hits, so P(<256 hits) ~ 1e-15 per row.  The host verifies the
device result is consistent (>= 256 hits, integral block sums, strictly
increasing positions) and falls back to an exact host computation
otherwise.

Device algorithm per core (one batch row per core, 8 cores), coarse/fine
two-level counting over a [32 blocks x 128 lane] transposed layout
(xT[f, p] = prefix[f*128 + p], prepared host-side by a free reshape):

  1. mask   mT = (xT <= 0.1)                    [32, 128] bf16   (VectorE)
  2. scan   CT = within-block inclusive cumsum  [32, 128] bf16   (VectorE)
     (CT <= 128, exact in bf16)
  3. block prefix: colsum = CT[:, -1];  two tiny PE matmuls with
     triangular constants give bc_incl/bc_excl [32, 1] each (fp32 PSUM)
  4. Cfull  CfT = CT + bc_excl  (VectorE add with PSUM per-partition
     scalar; values > 256 round in bf16 but stay > 255, so every
     comparison against thresholds <= 255 is still exact)
  5. coarse: S[f, j] = (bc_excl[f] < j <= bc_incl[f]) -- one-hot of the
     block holding hit j -- two VectorE ops against a j-iota, with the
     bc scalars read straight from PSUM
  6. fine, fully matmul-shaped so the result lands on partition 0:
       GT = CfT^T @ S            [128, 256] PSUM   (one PE matmul;
                                  GT[p, j] = C[p, F(j)], the gathered
                                  block column of hit j)
       IndT = (GT <= j-1)        [128, 256] bf16   (one VectorE
                                  tensor_tensor against an iota row)
       r    = ones^T @ IndT      [1, 256]  PSUM    (one PE matmul)
     pos(j) = 128*F(j) + r(j); no per-partition accumulators, no count
     transposes.
  7. bc_incl is PE-transposed next to r in the same PSUM row; a single
     ScalarE copy + one [1, 288] f32 DMA ships everything (one
     descriptor per SDMA engine -- a [128, x] output tile pays ~3.4 us
     of per-descriptor completion trickle on the final semaphore); host
     computes F(j) = #{f: bc_incl[f] <= j-1} and pos = 128*F + r.

All constants (iotas, triangular matrices, 32x32 identity, ones) are
generated on device with GpSimd iota + VectorE compares during the
input-DMA wait, so x is the only input DMA and the SDMA engines are
uncontended.  f32/bf16 iotas are exact here: every generated value is an
integer <= 256.
"""

import numpy as np

_THRES = np.float32(0.1)
_K = 256
_NB = 32            # blocks = SBUF partitions of the transposed layout
_BP = 128           # elements per block (free dim)
_P2 = _NB * _BP     # 4096: prefix elements scanned on device per row
_NCORES = 8

_NC_CACHE = {}


def _build_nc():
    import concourse.bacc as bacc
    import concourse.mybir as mybir
    from concourse.tile import TileContext

    dt = mybir.dt
    op = mybir.AluOpType
    act = mybir.ActivationFunctionType

    nc = bacc.Bacc(trn_type="TRN2", debug=False, enable_asserts=False)
    x = nc.dram_tensor("x", [_NB, _BP], dt.float32, kind="ExternalInput")
    out = nc.dram_tensor("out", [1, 288], dt.float32, kind="ExternalOutput")

    with TileContext(nc) as tc:
        with (
            tc.tile_pool(name="sb", bufs=1) as pool,
            tc.tile_pool(name="ps", bufs=1, space="PSUM") as psum,
        ):
            xt = pool.tile([_NB, _BP], dt.float32, tag="xt")
            nc.sync.dma_start(xt[:], x[:, :])

            # --- on-device constants (run during the x-DMA wait) ---
            z = pool.tile([_NB, _BP], dt.bfloat16, tag="z")
            nc.gpsimd.memset(z[:], 0.0)
            ones = pool.tile([128, 1], dt.bfloat16, tag="ones")
            nc.gpsimd.memset(ones[:], 1.0)
            # j-iota row 1..256 (exact in bf16: integers <= 256)
            jb = pool.tile([_NB, 256], dt.bfloat16, tag="jb")
            nc.gpsimd.iota(
                jb[:], [[1, 256]], base=1, channel_multiplier=0,
                allow_small_or_imprecise_dtypes=True,
            )
            # (j-1)-iota row 0..255 on all 128 partitions, f32
            jm1 = pool.tile([128, 256], dt.float32, tag="jm1")
            nc.gpsimd.iota(
                jm1[:], [[1, 256]], channel_multiplier=0,
                allow_small_or_imprecise_dtypes=True,
            )
            fcol = pool.tile([_NB, _NB], dt.float32, tag="fcol")
            nc.gpsimd.iota(
                fcol[:], [[1, _NB]], channel_multiplier=0,
                allow_small_or_imprecise_dtypes=True,
            )
            pif = pool.tile([_NB, 1], dt.float32, tag="pif")
            nc.gpsimd.iota(
                pif[:], [[1, 1]], channel_multiplier=1,
                allow_small_or_imprecise_dtypes=True,
            )
            tri_i = pool.tile([_NB, _NB], dt.bfloat16, tag="tri_i")
            nc.vector.tensor_scalar(tri_i[:], fcol[:], pif[:], None, op.is_ge)
            tri_x = pool.tile([_NB, _NB], dt.bfloat16, tag="tri_x")
            nc.vector.tensor_scalar(tri_x[:], fcol[:], pif[:], None, op.is_gt)
            idm32 = pool.tile([_NB, _NB], dt.float32, tag="idm32")
            nc.vector.tensor_scalar(idm32[:], fcol[:], pif[:], None, op.is_equal)

            # --- main pipeline ---
            # 1. mask
            mT = pool.tile([_NB, _BP], dt.bfloat16, tag="mT")
            nc.vector.tensor_scalar(mT[:], xt[:], float(_THRES), None, op.is_le)
            # 2. within-block inclusive scan (fp32 state, bf16 out, <=128)
            CT = pool.tile([_NB, _BP], dt.bfloat16, tag="CT")
            nc.vector.tensor_tensor_scan(
                CT[:], mT[:], z[:], 0.0, op.add, op.add
            )

            # 3. block prefix sums via triangular matmuls (N=1)
            psBC = psum.tile([_NB, 2], dt.float32, tag="psBC")
            nc.tensor.matmul(
                psBC[:, 0:1], tri_i[:], CT[:, 127:128], start=True, stop=True
            )
            nc.tensor.matmul(
                psBC[:, 1:2], tri_x[:], CT[:, 127:128], start=True, stop=True
            )

            # 4./5. Cfull and the S one-hot, bc scalars straight from PSUM
            CfT = pool.tile([_NB, _BP], dt.bfloat16, tag="CfT")
            nc.vector.tensor_scalar(
                CfT[:], CT[:], psBC[:, 1:2], None, op.add
            )
            t1 = pool.tile([_NB, 256], dt.bfloat16, tag="t1")
            nc.vector.tensor_scalar(
                t1[:], jb[:], psBC[:, 1:2], None, op.is_gt
            )
            Sf = pool.tile([_NB, 256], dt.bfloat16, tag="Sf")
            nc.vector.scalar_tensor_tensor(
                Sf[:], jb[:], psBC[:, 0:1], t1[:], op.is_le, op.mult
            )

            # bc_incl to SBUF for the transpose
            bcsb = pool.tile([_NB, 1], dt.float32, tag="bcsb")
            nc.scalar.activation(bcsb[:], psBC[:, 0:1], act.Copy)

            # outputs accumulate on partition 0 of one PSUM bank
            psR = psum.tile([1, 512], dt.float32, tag="psR")
            nc.tensor.transpose(
                psR[0:1, 256 : 256 + _NB], bcsb[:], idm32[:]
            )

            # 6. GT = CfT^T @ Sf, IndT = (GT <= j-1), r = ones^T @ IndT
            psGT = psum.tile([128, 256], dt.float32, tag="psGT")
            nc.tensor.matmul(psGT[:], CfT[:], Sf[:], start=True, stop=True)
            IndT = pool.tile([128, 256], dt.bfloat16, tag="IndT")
            nc.vector.tensor_tensor(IndT[:], psGT[:], jm1[:], op.is_le)
            nc.tensor.matmul(
                psR[0:1, 0:256], ones[:], IndT[:], start=True, stop=True
            )

            # 7. single copy + single-partition DMA out
            zf = pool.tile([1, 288], dt.float32, tag="zf")
            nc.scalar.activation(zf[:], psR[0:1, 0:288], act.Copy)
            nc.sync.dma_start(out[:, :], zf[:])
    nc.compile()
    return nc


def _get_nc():
    if "nc" not in _NC_CACHE:
        _NC_CACHE["nc"] = _build_nc()
    return _NC_CACHE["nc"]


def _decode_core(zf):
    """zf: [1, 288] f32 (r for j=1..256 | bc_incl[32]) -> [256] int64."""
    zf = zf.reshape(-1).astype(np.float64)
    r = zf[0:256]
    bc_incl = zf[256:288]
    if not (
        np.all(bc_incl == np.floor(bc_incl))
        and np.all(np.diff(bc_incl) >= 0)
        and bc_incl[-1] >= _K
        and np.all(r == np.floor(r))
        and r.min() >= 0
        and r.max() <= _BP - 1
    ):
        return None
    jm1 = np.arange(_K, dtype=np.float64)
    F = np.searchsorted(bc_incl, jm1, side="right")
    if F.max() >= _NB:
        return None
    pos = (_BP * F + r).astype(np.int64)
    if not (np.all(np.diff(pos) > 0) and pos[0] >= 0 and pos[-1] < _P2):
        return None
    return pos


def _run_device(prefix, trace=False):
    """prefix: [8, 4096] f32.  Returns (positions [8, 256] or None, results)."""
    from concourse.bass_utils import run_bass_kernel_spmd

    nc = _get_nc()
    in_maps = [
        {"x": np.ascontiguousarray(prefix[c].reshape(_NB, _BP))}
        for c in range(_NCORES)
    ]
    res = run_bass_kernel_spmd(
        nc, in_maps, core_ids=list(range(_NCORES)), trace=trace
    )
    pos = []
    for c in range(_NCORES):
        p = _decode_core(res.results[c]["out"])
        if p is None:
            return None, res
        pos.append(p)
    return np.stack(pos), res


def _host_row(flat_row):
    """Exact reference semantics for one row (fallback path)."""
    mask = flat_row <= _THRES
    hits = np.flatnonzero(mask)
    if hits.size >= _K:
        return hits[:_K].astype(np.int64)
    masked = np.where(flat_row > _THRES, flat_row, np.float32(0.0))
    order = np.argsort(masked, kind="stable")
    return order[:_K].astype(np.int64)


def kernel(confidence_map):
    cm = np.asarray(confidence_map)
    if cm.dtype != np.float32:
        cm = cm.astype(np.float32)
    B = cm.shape[0]
    num_tgt = cm.shape[2]
    flat = cm.reshape(B, -1)

    idx = None
    if B == _NCORES and flat.shape[1] >= _P2:
        idx, _ = _run_device(flat[:, :_P2])
    if idx is None:
        idx = np.stack([_host_row(flat[b]) for b in range(B)])

    src = (idx // num_tgt).astype(np.int32)
    tgt = (idx % num_tgt).astype(np.int32)
    return np.stack([src, tgt], axis=-1)
